# revision 7
# baseline (speedup 1.0000x reference)
"""Trainium2 Bass kernel for a single-head linear-projection attention block.

Reference computation (B=4, CH=256, N=4096):
    theta = Wt @ x        [B, 32, N]
    phi   = Wp @ x        [B, 32, N]
    g     = Wg @ x        [B, 128, N]
    scores = theta^T phi  [B, N, N]
    beta = softmax(scores, axis=-1)
    attn = g @ beta^T     [B, 128, N]
    out = gamma * (Wo @ attn) + x

Sharding: 8 cores = 4 batches x 2 query-halves. Each core owns one batch's
full sequence (for keys/values) and half the queries. The per-core x is
rotated so its query half is always columns 0:2048, keeping the SPMD program
identical across cores (softmax/attention are invariant to a consistent
permutation of the key axis). No collectives are needed.

V2 design (per core, all matmuls bf16 with fp32 PSUM accumulation). The
kernel is Scalar-engine (exp) bound: 8.4M exps/core at 1 elem/cycle/lane on
the only engine with transcendentals. Everything is organized to keep ACT
~100% busy on maximal-width EXP instructions:
  - PSUM: one [128, 3, 1024] score "ring" (6 banks; slot = global m-tile
    index % 3, one m-tile x 1024 queries per slot, written as two 512-col
    matmuls per the one-bank rule) + one rotating 2-bank psA slot
    (prologue projections/gT, attn accumulation, epilogue) = 8 banks.
  - 2 passes of 1024 queries; per pass 32 m-tiles processed as 16 pairs.
    A pair's K=32 score matmuls run concurrently in distinct PE row groups
    (phi/theta replicated to partitions 0:64) into two ring slots; ONE
    [128, 2x1024] strided-AP EXP consumes both -> ~(2048+400)/1.2GHz
    ~ 2.0us per pair vs 2 x 1.2us unpaired. Slot pairs rotate
    (0,1),(2,0),(1,2) as constant-stride 2-element APs (negative stride
    for the wrap pair). ACT reads may span banks; matmul writes may not.
  - attn accumulated over m into psA [128, 2, 512] (two query-half bank
    groups); softmax denominator via a bf16 tree: leaf folds on Vector,
    level-1 + fp32 chain on the otherwise-idle GpSimd (GpSimd has no PSUM
    port, so it only ever touches SBUF), partition-reduced with one
    ones-matmul.
  - psA is shared sequentially: pass-0 drips proj/gT blocks between its
    first iterations, pass-1 drips pass-0's epilogue; attn matmuls start
    deferred with an expt backlog (expt lives in SBUF, decoupling attn
    from the ring). The final epilogue is column-chunked to shorten the
    serial tail.
  - prologue: x split over 4 DMA queues (weights first on the gpsimd
    queue, replicas on sync), projections start as soon as column-block 0
    lands, and a dense warm-up burst trips the PE clock monitor (HAM) to
    full rate. First EXP issues ~12us in (vs ~26us in V1).
  - gamma folded into Wo on the host; fp32 residual add with x.
"""

import os
import sys

import numpy as np

B, CH, N = 4, 256, 4096
NCORES = 8
NH = N // 2   # queries per core
P = 128
MT = N // P   # 32 m-tiles
NQ = 1024     # queries per pass
NPASS = NH // NQ  # 2

_REPO_CANDIDATES = ["/opt/trn_rl_repo", "/root/.axon_site/_ro/trn_rl_repo"]


def _ensure_import_path():
    try:
        import concourse.bass  # noqa: F401
        return
    except ImportError:
        pass
    for cand in _REPO_CANDIDATES:
        if os.path.isdir(cand):
            sys.path.insert(0, cand)
            try:
                import concourse.bass  # noqa: F401
                return
            except ImportError:
                sys.path.pop(0)
    raise ImportError("could not locate concourse (bass) repo")


_CACHE = {}


def build_bass():
    """Build + compile the per-core Tile program (identical on all 8 cores)."""
    _ensure_import_path()
    import concourse.bacc as bacc
    import concourse.tile as tile
    from concourse import mybir

    dt = mybir.dt
    f32 = dt.float32
    bf16 = dt.bfloat16
    Exp = mybir.ActivationFunctionType.Exp

    nc = bacc.Bacc(
        "TRN2",
        target_bir_lowering=False,
        debug=False,
        num_devices=NCORES,
    )

    # Per-core DRAM I/O.
    x_d = nc.dram_tensor("x", [CH, N], bf16, kind="ExternalInput")
    xq_d = nc.dram_tensor("xq", [CH, NH], f32, kind="ExternalInput")
    wt_d = nc.dram_tensor("wt", [CH, 32], bf16, kind="ExternalInput")   # Wt^T
    wp_d = nc.dram_tensor("wp", [CH, 32], bf16, kind="ExternalInput")   # Wp^T
    wg_d = nc.dram_tensor("wg", [CH, 128], bf16, kind="ExternalInput")  # Wg^T
    wo_d = nc.dram_tensor("wo", [128, CH], bf16, kind="ExternalInput")  # (gamma*Wo)^T
    out_d = nc.dram_tensor("out", [CH, NH], f32, kind="ExternalOutput")

    with tile.TileContext(nc) as tc:
        with (
            tc.tile_pool(name="const", bufs=1) as const,
            tc.tile_pool(name="xp", bufs=1) as xp,
            tc.tile_pool(name="proj", bufs=1) as proj,
            tc.tile_pool(name="expp", bufs=10) as expp,
            tc.tile_pool(name="tree", bufs=3) as tree,
            tc.tile_pool(name="acc", bufs=2) as acc,
            tc.tile_pool(name="outp", bufs=1) as outp,
            tc.tile_pool(name="ringp", bufs=1, space="PSUM") as ringp,
            tc.tile_pool(name="psA", bufs=1, space="PSUM") as psA,
        ):
            # ---- tiny weights first, on the gpsimd HWDGE queue ----
            wt_sb = const.tile([P, 2, 32], bf16)
            wp_sb = const.tile([P, 2, 32], bf16)
            wg_sb = const.tile([P, 2, 128], bf16)
            wo_sb = const.tile([P, CH], bf16)
            ones_sb = const.tile([P, P], bf16)
            warm_sb = const.tile([P, 512], bf16)
            nc.gpsimd.dma_start(
                out=wt_sb, in_=wt_d.ap().rearrange("(kb p) m -> p kb m", p=P)
            )
            nc.gpsimd.dma_start(
                out=wp_sb, in_=wp_d.ap().rearrange("(kb p) m -> p kb m", p=P)
            )
            nc.gpsimd.dma_start(
                out=wg_sb, in_=wg_d.ap().rearrange("(kb p) m -> p kb m", p=P)
            )
            nc.vector.memset(ones_sb, 1.0)
            nc.vector.memset(warm_sb, 0.0)

            # ---- x column blocks across 4 queues; cb0 (both kb) first ----
            x_sb = xp.tile([P, 2, N], bf16)
            xq_sb = xp.tile([P, 2, NH], f32)

            def dma_x(eng, kb, cb):
                eng.dma_start(
                    out=x_sb[:, kb, cb * 1024:(cb + 1) * 1024],
                    in_=x_d[kb * P:(kb + 1) * P, cb * 1024:(cb + 1) * 1024],
                )

            for cb in range(4):
                dma_x(nc.sync, 0, cb)
                dma_x(nc.scalar, 1, cb)
            nc.gpsimd.dma_start(out=wo_sb, in_=wo_d.ap())
            # residual slice: only needed by the epilogues (first ~45us in);
            # gpsimd queue so its long transfer never delays x or replicas
            for kb in range(2):
                nc.gpsimd.dma_start(
                    out=xq_sb[:, kb, :], in_=xq_d[kb * P:(kb + 1) * P, :]
                )

            # ---- PSUM: 3-slot score ring (6 banks) + psA (2 banks) ----
            ring = ringp.tile([P, 3, NQ], f32)

            # dense dummy matmul burst during the initial x-DMA wait: trips
            # the PE clock monitor (HAM) to full rate before the projections
            for _ in range(12):
                nc.tensor.matmul(
                    ring[:, 2, 0:512], lhsT=ones_sb, rhs=warm_sb,
                    start=True, stop=True,
                )

            # ---- projections: theta/phi column-packed per 1024-col block
            # (theta at psum partitions 0:32, phi at 32:64), one cast each,
            # then replica DMAs (sync queue) fill the opposite 32-partition
            # group so both live at 0:64 for 2x row-group score packing. ----
            theta_sb = proj.tile([64, NH], bf16)
            phi_sb = proj.tile([64, N], bf16)
            gT_sb = proj.tile([P, MT, P], bf16)

            def emit_proj(cb):
                cbs = slice(cb * 1024, (cb + 1) * 1024)
                ps_p = psA.tile([64, 2, 512], f32, tag="ps")
                for h in range(2):
                    chs = slice(cb * 1024 + h * 512, cb * 1024 + (h + 1) * 512)
                    for kb in range(2):
                        if cb < 2:
                            nc.tensor.matmul(
                                ps_p[0:32, h, :],
                                lhsT=wt_sb[:, kb, :],
                                rhs=x_sb[:, kb, chs],
                                start=(kb == 0),
                                stop=(kb == 1),
                                skip_group_check=True,
                            )
                        nc.tensor.matmul(
                            ps_p[32:64, h, :],
                            lhsT=wp_sb[:, kb, :],
                            rhs=x_sb[:, kb, chs],
                            start=(kb == 0),
                            stop=(kb == 1),
                            skip_group_check=True,
                        )
                if cb < 2:
                    nc.vector.tensor_copy(
                        out=theta_sb[0:32, cbs].rearrange("p (h c) -> p h c", h=2),
                        in_=ps_p[0:32, :, :],
                    )
                    nc.sync.dma_start(
                        out=theta_sb[32:64, cbs], in_=theta_sb[0:32, cbs]
                    )
                nc.vector.tensor_copy(
                    out=phi_sb[32:64, cbs].rearrange("p (h c) -> p h c", h=2),
                    in_=ps_p[32:64, :, :],
                )
                nc.sync.dma_start(out=phi_sb[0:32, cbs], in_=phi_sb[32:64, cbs])

            def emit_gt_group(grp):
                # gT[m, c] for 8 m-tiles; cast back to SBUF on Vector
                # (GpSimd cannot read PSUM)
                ps_g = psA.tile([P, 8, P], f32, tag="ps")
                for j in range(8):
                    mt = grp * 8 + j
                    for kb in range(2):
                        nc.tensor.matmul(
                            ps_g[:, j, :],
                            lhsT=x_sb[:, kb, mt * P:(mt + 1) * P],
                            rhs=wg_sb[:, kb, :],
                            start=(kb == 0),
                            stop=(kb == 1),
                        )
                nc.vector.tensor_copy(
                    out=gT_sb[:, grp * 8:(grp + 1) * 8, :], in_=ps_g
                )

            # block 0 before the loop: everything iteration 0 needs
            emit_proj(0)
            emit_gt_group(0)

            out_sb = outp.tile([P, 2, NH], f32)

            # ---------------- epilogue pieces (generator) ----------------
            def epilogue_pieces(nh, attn_ps, root_bf, nchunk=1):
                """Normalization + output projection for pass nh. Piece 0
                (the cast) frees the attn psA slot; later pieces reuse the
                psA banks sequentially. root_bf: [P, NQ] bf16 per-partition
                partial denominators (pre partition-reduction)."""
                A_bf = acc.tile([P, NQ], bf16, tag="abf")
                nc.vector.tensor_copy(
                    out=A_bf.rearrange("p (h c) -> p h c", h=2), in_=attn_ps
                )
                yield
                ps_S = psA.tile([P, 2, 512], f32, tag="ps")
                for h in range(2):
                    nc.tensor.matmul(
                        ps_S[:, h, :],
                        lhsT=ones_sb,
                        rhs=root_bf[:, h * 512:(h + 1) * 512],
                        start=True,
                        stop=True,
                        skip_group_check=True,
                    )
                yield
                recip = acc.tile([P, NQ], f32, tag="recip")
                nc.vector.reciprocal_approx_fast(
                    out=recip.rearrange("p (h c) -> p h c", h=2), in_=ps_S
                )
                yield
                CW = 512
                for ck in range(NQ // CW):
                    cks = slice(ck * CW, (ck + 1) * CW)
                    gks = slice(nh * NQ + ck * CW, nh * NQ + (ck + 1) * CW)
                    ps_o = psA.tile([P, 2, 512], f32, tag="ps")
                    for oc in range(2):
                        nc.tensor.matmul(
                            ps_o[:, oc, :],
                            lhsT=wo_sb[:, oc * P:(oc + 1) * P],
                            rhs=A_bf[:, cks],
                            start=True,
                            stop=True,
                            skip_group_check=True,
                        )
                    for oc in range(2):
                        tmp = acc.tile([P, CW], f32, tag=f"tmp{oc}")
                        nc.vector.tensor_mul(tmp, ps_o[:, oc, :], recip[:, cks])
                        nc.gpsimd.tensor_add(
                            out_sb[:, oc, gks], tmp, xq_sb[:, oc, gks]
                        )
                        nc.sync.dma_start(
                            out=out_d[oc * P:(oc + 1) * P, gks],
                            in_=out_sb[:, oc, gks],
                        )
                        if nchunk > 1:
                            yield
                    if nchunk == 1 and ck == NQ // CW - 1:
                        yield

            # -------------------- main loop: 2 passes --------------------
            # psA slot owners are strictly sequential: proj0,gT0 (above),
            # then pass-0 drips proj1,gT1,...,proj3,gT3, then attn(p0),
            # then pass-0's epilogue dripped into pass 1, then attn(p1), ...
            pending = None
            for nh in range(NPASS):
                qs = nh * NQ
                if nh == 0:
                    drip = []
                    for cb in range(1, 4):
                        drip.append(lambda c=cb: emit_proj(c))
                        drip.append(lambda c=cb: emit_gt_group(c))
                else:
                    drip = None  # pass>0: drip = previous epilogue pieces
                attn_ps = None
                attn_cnt = 0
                attn_backlog = []  # (mt, expt_tile, half) awaiting emission
                quad = None
                chain = None
                for k in range(MT // 2):
                    mtE, mtO = 2 * k, 2 * k + 1
                    tg = nh * MT + 2 * k  # global tile counter
                    sE, sO = tg % 3, (tg + 1) % 3
                    # --- score matmuls (concurrent PE row groups), two
                    #     512-col query halves per slot (one-bank rule) ---
                    for h in range(2):
                        for j, (mt, s) in enumerate(((mtE, sE), (mtO, sO))):
                            nc.tensor.matmul(
                                ring[:, s, h * 512:(h + 1) * 512],
                                lhsT=phi_sb[
                                    32 * j:32 * (j + 1), mt * P:(mt + 1) * P
                                ],
                                rhs=theta_sb[
                                    32 * j:32 * (j + 1),
                                    qs + h * 512:qs + (h + 1) * 512,
                                ],
                                start=True,
                                stop=True,
                                skip_group_check=True,
                            )
                    # --- exp over BOTH slots in one strided instruction ---
                    expt = expp.tile([P, 2, NQ], bf16, tag="expt")
                    if sO == sE + 1:
                        pair_in = ring[:, sE:sE + 2, :]
                    else:  # (2, 0): 2-element dim with negative stride
                        pair_in = ring[:, sE::-2, :]
                    nc.scalar.activation(out=expt, in_=pair_in, func=Exp)
                    attn_backlog.append((mtE, expt, 0))
                    attn_backlog.append((mtO, expt, 1))
                    # --- drip prologue blocks / previous pass's epilogue;
                    #     attn matmuls open once psA's queue frees up ---
                    if drip:
                        drip.pop(0)()
                    elif pending is not None:
                        if next(pending, "done") == "done":
                            pending = None
                    if not drip and pending is None:
                        if attn_ps is None:
                            attn_ps = psA.tile([P, 2, 512], f32, tag="ps")
                        for mt, et, half in attn_backlog:
                            for h in range(2):
                                nc.tensor.matmul(
                                    attn_ps[:, h, :],
                                    lhsT=gT_sb[:, mt, :],
                                    rhs=et[:, half, h * 512:(h + 1) * 512],
                                    start=(attn_cnt == 0),
                                    stop=(attn_cnt == MT - 1),
                                    skip_group_check=True,
                                )
                            attn_cnt += 1
                        attn_backlog = []
                    # --- denominator tree: leaf folds on Vector, level-1
                    #     and fp32 chain on GpSimd (SBUF-only), bf16 root ---
                    leaf = tree.tile([P, NQ], bf16, tag="leaf")
                    nc.vector.tensor_add(leaf, expt[:, 0, :], expt[:, 1, :])
                    if quad is None:
                        quad = leaf
                    else:
                        node = tree.tile([P, NQ], bf16, tag="lvl1")
                        nc.gpsimd.tensor_add(node, quad, leaf)
                        quad = None
                        last = k == MT // 2 - 1
                        if chain is None:
                            chain = node
                        else:
                            nt = tree.tile(
                                [P, NQ], bf16 if last else f32,
                                tag="sroot" if last else "chain",
                            )
                            nc.gpsimd.tensor_add(nt, chain, node)
                            chain = nt
                assert quad is None and not attn_backlog and attn_cnt == MT
                assert pending is None and not drip
                pending = epilogue_pieces(
                    nh, attn_ps, chain, nchunk=1 if nh < NPASS - 1 else 2
                )
            for _ in pending:
                pass

    nc.compile()
    return nc


def get_nc():
    if "nc" not in _CACHE:
        _CACHE["nc"] = build_bass()
    return _CACHE["nc"]


def make_in_maps(x, Wt, Wp, Wg, Wo, gamma):
    import ml_dtypes

    bf16 = ml_dtypes.bfloat16
    x = np.asarray(x, dtype=np.float32)
    wt = np.ascontiguousarray(np.asarray(Wt, np.float32).T).astype(bf16)
    wp = np.ascontiguousarray(np.asarray(Wp, np.float32).T).astype(bf16)
    wg = np.ascontiguousarray(np.asarray(Wg, np.float32).T).astype(bf16)
    wo = np.ascontiguousarray(
        (float(np.asarray(gamma)) * np.asarray(Wo, np.float32)).T
    ).astype(bf16)
    in_maps = []
    for i in range(NCORES):
        b, h = divmod(i, 2)
        xb = x[b]
        if h:
            xb = np.concatenate([xb[:, NH:], xb[:, :NH]], axis=1)
        in_maps.append(
            {
                "x": np.ascontiguousarray(xb).astype(bf16),
                "xq": np.ascontiguousarray(x[b][:, h * NH:(h + 1) * NH]),
                "wt": wt,
                "wp": wp,
                "wg": wg,
                "wo": wo,
            }
        )
    return in_maps


def gather_out(results):
    out = np.empty((B, CH, N), np.float32)
    for i in range(NCORES):
        b, h = divmod(i, 2)
        out[b][:, h * NH:(h + 1) * NH] = results[i]["out"]
    return out


def kernel(x, Wt, Wp, Wg, Wo, gamma):
    _ensure_import_path()
    from concourse.bass_utils import run_bass_kernel_spmd

    nc = get_nc()
    in_maps = make_in_maps(x, Wt, Wp, Wg, Wo, gamma)
    res = run_bass_kernel_spmd(nc, in_maps, core_ids=list(range(NCORES)))
    return gather_out(res.results)


# revision 17
# speedup vs baseline: 1.1558x; 1.1558x over previous
"""Trainium2 Bass kernel for a single-head linear-projection attention block.

Reference computation (B=4, CH=256, N=4096):
    theta = Wt @ x        [B, 32, N]
    phi   = Wp @ x        [B, 32, N]
    g     = Wg @ x        [B, 128, N]
    scores = theta^T phi  [B, N, N]
    beta = softmax(scores, axis=-1)
    attn = g @ beta^T     [B, 128, N]
    out = gamma * (Wo @ attn) + x

Sharding: 8 cores = 4 batches x 2 query-halves. Each core owns one batch's
full sequence (for keys/values) and half the queries. The per-core x is
rotated so its query half is always columns 0:2048, keeping the SPMD program
identical across cores (softmax/attention are invariant to a consistent
permutation of the key axis). No collectives are needed.

V2 design (per core, all matmuls bf16 with fp32 PSUM accumulation). The
kernel is Scalar-engine (exp) bound: 8.4M exps/core at 1 elem/cycle/lane on
the only engine with transcendentals. Everything is organized to keep ACT
~100% busy on maximal-width EXP instructions:
  - PSUM: one [128, 3, 1024] score "ring" (6 banks; slot = global m-tile
    index % 3, one m-tile x 1024 queries per slot, written as two 512-col
    matmuls per the one-bank rule) + one rotating 2-bank psA slot
    (prologue projections/gT, attn accumulation, epilogue) = 8 banks.
  - 2 passes of 1024 queries; per pass 32 m-tiles processed as 16 pairs.
    A pair's K=32 score matmuls run concurrently in distinct PE row groups
    (phi/theta replicated to partitions 0:64) into two ring slots; ONE
    [128, 2x1024] strided-AP EXP consumes both -> ~(2048+400)/1.2GHz
    ~ 2.0us per pair vs 2 x 1.2us unpaired. Slot pairs rotate
    (0,1),(2,0),(1,2) as constant-stride 2-element APs (negative stride
    for the wrap pair). ACT reads may span banks; matmul writes may not.
  - attn accumulated over m into psA [128, 2, 512] (two query-half bank
    groups); softmax denominator via a bf16 tree: leaf folds on Vector,
    level-1 + fp32 chain on the otherwise-idle GpSimd (GpSimd has no PSUM
    port, so it only ever touches SBUF), partition-reduced with one
    ones-matmul.
  - psA is shared sequentially: pass-0 drips proj/gT blocks between its
    first iterations, pass-1 drips pass-0's epilogue; attn matmuls start
    deferred with an expt backlog (expt lives in SBUF, decoupling attn
    from the ring). The final epilogue is column-chunked to shorten the
    serial tail.
  - prologue: x split over 4 DMA queues (weights first on the gpsimd
    queue, replicas on sync), projections start as soon as column-block 0
    lands, and a dense warm-up burst trips the PE clock monitor (HAM) to
    full rate. First EXP issues ~12us in (vs ~26us in V1).
  - gamma folded into Wo on the host; fp32 residual add with x.
"""

import os
import sys

import numpy as np

B, CH, N = 4, 256, 4096
NCORES = 8
NH = N // 2   # queries per core
P = 128
MT = N // P   # 32 m-tiles
NQ = 1024     # queries per pass
NPASS = NH // NQ  # 2

_REPO_CANDIDATES = ["/opt/trn_rl_repo", "/root/.axon_site/_ro/trn_rl_repo"]


def _ensure_import_path():
    try:
        import concourse.bass  # noqa: F401
        return
    except ImportError:
        pass
    for cand in _REPO_CANDIDATES:
        if os.path.isdir(cand):
            sys.path.insert(0, cand)
            try:
                import concourse.bass  # noqa: F401
                return
            except ImportError:
                sys.path.pop(0)
    raise ImportError("could not locate concourse (bass) repo")


_CACHE = {}


def build_bass():
    """Build + compile the per-core Tile program (identical on all 8 cores)."""
    _ensure_import_path()
    import concourse.bacc as bacc
    import concourse.tile as tile
    from concourse import mybir

    dt = mybir.dt
    f32 = dt.float32
    bf16 = dt.bfloat16
    Exp = mybir.ActivationFunctionType.Exp

    nc = bacc.Bacc(
        "TRN2",
        target_bir_lowering=False,
        debug=False,
        num_devices=NCORES,
    )

    # Per-core DRAM I/O.
    x_d = nc.dram_tensor("x", [CH, N], bf16, kind="ExternalInput")
    xq_d = nc.dram_tensor("xq", [CH, NH], f32, kind="ExternalInput")
    wt_d = nc.dram_tensor("wt", [CH, 32], bf16, kind="ExternalInput")   # Wt^T
    wp_d = nc.dram_tensor("wp", [CH, 32], bf16, kind="ExternalInput")   # Wp^T
    wg_d = nc.dram_tensor("wg", [CH, 128], bf16, kind="ExternalInput")  # Wg^T
    wo_d = nc.dram_tensor("wo", [128, CH], bf16, kind="ExternalInput")  # (gamma*Wo)^T
    out_d = nc.dram_tensor("out", [CH, NH], f32, kind="ExternalOutput")

    with tile.TileContext(nc) as tc:
        with (
            tc.tile_pool(name="const", bufs=1) as const,
            tc.tile_pool(name="xp", bufs=1) as xp,
            tc.tile_pool(name="proj", bufs=1) as proj,
            tc.tile_pool(name="expp", bufs=14) as expp,
            tc.tile_pool(name="tree", bufs=6) as tree,
            tc.tile_pool(name="acc", bufs=2) as acc,
            tc.tile_pool(name="outp", bufs=1) as outp,
            tc.tile_pool(name="ringp", bufs=1, space="PSUM") as ringp,
            tc.tile_pool(name="psA", bufs=1, space="PSUM") as psA,
        ):
            # ---- tiny weights first, on the gpsimd HWDGE queue ----
            wt_sb = const.tile([P, 2, 32], bf16)
            wp_sb = const.tile([P, 2, 32], bf16)
            wg_sb = const.tile([P, 2, 128], bf16)
            wo_sb = const.tile([P, CH], bf16)
            ones_sb = const.tile([P, P], bf16)
            warm_sb = const.tile([P, 512], bf16)
            nc.gpsimd.dma_start(
                out=wt_sb, in_=wt_d.ap().rearrange("(kb p) m -> p kb m", p=P)
            )
            nc.gpsimd.dma_start(
                out=wp_sb, in_=wp_d.ap().rearrange("(kb p) m -> p kb m", p=P)
            )
            nc.gpsimd.dma_start(
                out=wg_sb, in_=wg_d.ap().rearrange("(kb p) m -> p kb m", p=P)
            )
            nc.vector.memset(ones_sb, 1.0)
            nc.vector.memset(warm_sb, 0.0)

            # ---- x column blocks across 4 queues; cb0 (both kb) first ----
            x_sb = xp.tile([P, 2, N], bf16)
            xq_sb = xp.tile([P, 2, NH], f32)

            def dma_x(eng, kb, cb):
                eng.dma_start(
                    out=x_sb[:, kb, cb * 1024:(cb + 1) * 1024],
                    in_=x_d[kb * P:(kb + 1) * P, cb * 1024:(cb + 1) * 1024],
                )

            for cb in range(4):
                dma_x(nc.sync, 0, cb)
                dma_x(nc.scalar, 1, cb)
            nc.gpsimd.dma_start(out=wo_sb, in_=wo_d.ap())
            # residual slice: only needed by the epilogues (first ~45us in);
            # gpsimd queue so its long transfer never delays x or replicas
            for kb in range(2):
                nc.gpsimd.dma_start(
                    out=xq_sb[:, kb, :], in_=xq_d[kb * P:(kb + 1) * P, :]
                )

            # ---- PSUM: 3-slot fp32 score ring (6 banks; slot = one m-tile
            # x 1024 queries, written as two 512-col matmuls per the
            # one-bank rule, consumed by one single-slot EXP) + psA ----
            ring = ringp.tile([P, 3, NQ], f32)

            # dense dummy matmul burst during the initial x-DMA wait: trips
            # the PE clock monitor (HAM) to full rate before the projections
            for _ in range(12):
                nc.tensor.matmul(
                    ring[:, 2, 0:512], lhsT=ones_sb, rhs=warm_sb,
                    start=True, stop=True,
                )

            # ---- projections: theta/phi column-packed per 1024-col block
            # (theta at psum partitions 0:32, phi at 32:64), one cast each,
            # then replica DMAs (sync queue) fill the opposite 32-partition
            # group so both live at 0:64 for 2x row-group score packing. ----
            theta_sb = proj.tile([64, NH], bf16)
            phi_sb = proj.tile([64, N], bf16)
            gT_sb = proj.tile([P, MT, P], bf16)

            def emit_proj(cb):
                cbs = slice(cb * 1024, (cb + 1) * 1024)
                ps_p = psA.tile([64, 2, 512], f32, tag="ps")
                for h in range(2):
                    chs = slice(cb * 1024 + h * 512, cb * 1024 + (h + 1) * 512)
                    for kb in range(2):
                        if cb < 2:
                            nc.tensor.matmul(
                                ps_p[0:32, h, :],
                                lhsT=wt_sb[:, kb, :],
                                rhs=x_sb[:, kb, chs],
                                start=(kb == 0),
                                stop=(kb == 1),
                                skip_group_check=True,
                            )
                        nc.tensor.matmul(
                            ps_p[32:64, h, :],
                            lhsT=wp_sb[:, kb, :],
                            rhs=x_sb[:, kb, chs],
                            start=(kb == 0),
                            stop=(kb == 1),
                            skip_group_check=True,
                        )
                if cb < 2:
                    nc.vector.tensor_copy(
                        out=theta_sb[0:32, cbs].rearrange("p (h c) -> p h c", h=2),
                        in_=ps_p[0:32, :, :],
                    )
                    nc.sync.dma_start(
                        out=theta_sb[32:64, cbs], in_=theta_sb[0:32, cbs]
                    )
                nc.vector.tensor_copy(
                    out=phi_sb[32:64, cbs].rearrange("p (h c) -> p h c", h=2),
                    in_=ps_p[32:64, :, :],
                )
                nc.sync.dma_start(out=phi_sb[0:32, cbs], in_=phi_sb[32:64, cbs])

            def emit_gt_group(grp):
                # gT[m, c] for 8 m-tiles; cast back to SBUF on Vector
                # (GpSimd cannot read PSUM)
                ps_g = psA.tile([P, 8, P], f32, tag="ps")
                for j in range(8):
                    mt = grp * 8 + j
                    for kb in range(2):
                        nc.tensor.matmul(
                            ps_g[:, j, :],
                            lhsT=x_sb[:, kb, mt * P:(mt + 1) * P],
                            rhs=wg_sb[:, kb, :],
                            start=(kb == 0),
                            stop=(kb == 1),
                        )
                nc.vector.tensor_copy(
                    out=gT_sb[:, grp * 8:(grp + 1) * 8, :], in_=ps_g
                )

            # block 0 before the loop: everything iteration 0 needs
            emit_proj(0)
            emit_gt_group(0)

            out_sb = outp.tile([P, 2, NH], f32)

            # ---------------- epilogue pieces (generator) ----------------
            def epilogue_pieces(nh, attn_ps, root_bf, nchunk=1):
                """Normalization + output projection for pass nh. Piece 0
                (the cast) frees the attn psA slot; later pieces reuse the
                psA banks sequentially. root_bf: [P, NQ] bf16 per-partition
                partial denominators (pre partition-reduction)."""
                A_bf = acc.tile([P, NQ], bf16, tag="abf")
                nc.vector.tensor_copy(
                    out=A_bf.rearrange("p (h c) -> p h c", h=2), in_=attn_ps
                )
                yield
                ps_S = psA.tile([P, 2, 512], f32, tag="ps")
                for h in range(2):
                    nc.tensor.matmul(
                        ps_S[:, h, :],
                        lhsT=ones_sb,
                        rhs=root_bf[:, h * 512:(h + 1) * 512],
                        start=True,
                        stop=True,
                        skip_group_check=True,
                    )
                yield
                recip = acc.tile([P, NQ], f32, tag="recip")
                nc.vector.reciprocal_approx_fast(
                    out=recip.rearrange("p (h c) -> p h c", h=2), in_=ps_S
                )
                yield
                CW = 512
                for ck in range(NQ // CW):
                    cks = slice(ck * CW, (ck + 1) * CW)
                    gks = slice(nh * NQ + ck * CW, nh * NQ + (ck + 1) * CW)
                    ps_o = psA.tile([P, 2, 512], f32, tag="ps")
                    for oc in range(2):
                        nc.tensor.matmul(
                            ps_o[:, oc, :],
                            lhsT=wo_sb[:, oc * P:(oc + 1) * P],
                            rhs=A_bf[:, cks],
                            start=True,
                            stop=True,
                            skip_group_check=True,
                        )
                    for oc in range(2):
                        tmp = acc.tile([P, CW], f32, tag=f"tmp{oc}")
                        nc.vector.tensor_mul(tmp, ps_o[:, oc, :], recip[:, cks])
                        nc.gpsimd.tensor_add(
                            out_sb[:, oc, gks], tmp, xq_sb[:, oc, gks]
                        )
                        nc.sync.dma_start(
                            out=out_d[oc * P:(oc + 1) * P, gks],
                            in_=out_sb[:, oc, gks],
                        )
                        if nchunk > 1:
                            yield
                    if nchunk == 1 and ck == NQ // CW - 1:
                        yield

            # -------------------- main loop: 2 passes --------------------
            # psA slot owners are strictly sequential: proj0,gT0 (above),
            # then pass-0 drips proj1,gT1,...,proj3,gT3, then attn(p0),
            # then pass-0's epilogue dripped into pass 1, then attn(p1), ...
            pending = None
            leftover = []  # previous pass's lagged attn matmuls
            for nh in range(NPASS):
                qs = nh * NQ
                if nh == 0:
                    drip = []
                    for cb in range(1, 4):
                        drip.append(lambda c=cb: emit_proj(c))
                        drip.append(lambda c=cb: emit_gt_group(c))
                else:
                    drip = None  # pass>0: drip = previous epilogue pieces

                def flush_prev_attn():
                    # lagged attn matmuls of the PREVIOUS pass: their exps
                    # are long done, so these never stall the PE queue
                    nonlocal leftover
                    for mt, et, aps, st, sp in leftover:
                        for h in range(2):
                            nc.tensor.matmul(
                                aps[:, h, :],
                                lhsT=gT_sb[:, mt, :],
                                rhs=et[:, h * 512:(h + 1) * 512],
                                start=st,
                                stop=sp,
                                skip_group_check=True,
                            )
                    leftover = []

                attn_ps = None
                attn_cnt = 0
                attn_backlog = []  # (mt, expt_tile, half) awaiting emission
                quad = None
                chain = None
                for k in range(MT // 2):
                    mtE, mtO = 2 * k, 2 * k + 1
                    tg = nh * MT + 2 * k  # global tile counter
                    sE, sO = tg % 3, (tg + 1) % 3
                    # --- score matmuls: two 512-col halves per m-tile (the
                    #     one-bank rule); the two m-tiles of the pair run in
                    #     concurrent PE row groups ---
                    for h in range(2):
                        for j, (mt, s) in enumerate(((mtE, sE), (mtO, sO))):
                            nc.tensor.matmul(
                                ring[:, s, h * 512:(h + 1) * 512],
                                lhsT=phi_sb[
                                    32 * j:32 * (j + 1), mt * P:(mt + 1) * P
                                ],
                                rhs=theta_sb[
                                    32 * j:32 * (j + 1),
                                    qs + h * 512:qs + (h + 1) * 512,
                                ],
                                start=True,
                                stop=True,
                                skip_group_check=True,
                            )
                    if k == 0:
                        flush_prev_attn()
                    # --- one single-slot exp per m-tile (the 3-slot ring
                    #     keeps the producer ~2 tiles ahead) ---
                    etE = expp.tile([P, NQ], bf16, tag="expt")
                    nc.scalar.activation(out=etE, in_=ring[:, sE, :], func=Exp)
                    etO = expp.tile([P, NQ], bf16, tag="expt")
                    nc.scalar.activation(out=etO, in_=ring[:, sO, :], func=Exp)
                    attn_backlog.append((mtE, etE))
                    attn_backlog.append((mtO, etO))
                    # --- drip prologue blocks / previous pass's epilogue;
                    #     attn matmuls open once psA's queue frees up, and
                    #     keep two tiles of lag so they never wait on the
                    #     exp that was just issued ---
                    if drip:
                        drip.pop(0)()
                    elif pending is not None:
                        if next(pending, "done") == "done":
                            pending = None
                    if not drip and pending is None:
                        if attn_ps is None:
                            attn_ps = psA.tile([P, 2, 512], f32, tag="ps")
                        while len(attn_backlog) > 2:
                            mt, et = attn_backlog.pop(0)
                            for h in range(2):
                                nc.tensor.matmul(
                                    attn_ps[:, h, :],
                                    lhsT=gT_sb[:, mt, :],
                                    rhs=et[:, h * 512:(h + 1) * 512],
                                    start=(attn_cnt == 0),
                                    stop=False,
                                    skip_group_check=True,
                                )
                            attn_cnt += 1
                    # --- denominator tree: leaf folds on Vector, level-1
                    #     and fp32 chain on GpSimd (SBUF-only), bf16 root ---
                    leaf = tree.tile([P, NQ], bf16, tag="leaf")
                    nc.vector.tensor_add(leaf, etE, etO)
                    if quad is None:
                        quad = leaf
                    else:
                        node = tree.tile([P, NQ], bf16, tag="lvl1")
                        nc.vector.tensor_add(node, quad, leaf)
                        quad = None
                        last = k == MT // 2 - 1
                        if chain is None:
                            chain = node
                        else:
                            nt = tree.tile(
                                [P, NQ], bf16 if last else f32,
                                tag="sroot" if last else "chain",
                            )
                            nc.gpsimd.tensor_add(nt, chain, node)
                            chain = nt
                # hand this pass's lagged attn tail to the next pass (or to
                # the post-loop flush for the final pass)
                assert quad is None and pending is None and not drip
                assert len(attn_backlog) == 2 and attn_cnt == MT - 2
                for i, (mt, et) in enumerate(attn_backlog):
                    leftover.append((mt, et, attn_ps, False, i == 1))
                pending = epilogue_pieces(
                    nh, attn_ps, chain, nchunk=1 if nh < NPASS - 1 else 2
                )
            flush_prev_attn()
            for _ in pending:
                pass

    nc.compile()
    return nc


def get_nc():
    if "nc" not in _CACHE:
        _CACHE["nc"] = build_bass()
    return _CACHE["nc"]


def make_in_maps(x, Wt, Wp, Wg, Wo, gamma):
    import ml_dtypes

    bf16 = ml_dtypes.bfloat16
    x = np.asarray(x, dtype=np.float32)
    wt = np.ascontiguousarray(np.asarray(Wt, np.float32).T).astype(bf16)
    wp = np.ascontiguousarray(np.asarray(Wp, np.float32).T).astype(bf16)
    wg = np.ascontiguousarray(np.asarray(Wg, np.float32).T).astype(bf16)
    wo = np.ascontiguousarray(
        (float(np.asarray(gamma)) * np.asarray(Wo, np.float32)).T
    ).astype(bf16)
    in_maps = []
    for i in range(NCORES):
        b, h = divmod(i, 2)
        xb = x[b]
        if h:
            xb = np.concatenate([xb[:, NH:], xb[:, :NH]], axis=1)
        in_maps.append(
            {
                "x": np.ascontiguousarray(xb).astype(bf16),
                "xq": np.ascontiguousarray(x[b][:, h * NH:(h + 1) * NH]),
                "wt": wt,
                "wp": wp,
                "wg": wg,
                "wo": wo,
            }
        )
    return in_maps


def gather_out(results):
    out = np.empty((B, CH, N), np.float32)
    for i in range(NCORES):
        b, h = divmod(i, 2)
        out[b][:, h * NH:(h + 1) * NH] = results[i]["out"]
    return out


def kernel(x, Wt, Wp, Wg, Wo, gamma):
    _ensure_import_path()
    from concourse.bass_utils import run_bass_kernel_spmd

    nc = get_nc()
    in_maps = make_in_maps(x, Wt, Wp, Wg, Wo, gamma)
    res = run_bass_kernel_spmd(nc, in_maps, core_ids=list(range(NCORES)))
    return gather_out(res.results)


# revision 18
# speedup vs baseline: 1.1796x; 1.0207x over previous
"""Trainium2 Bass kernel for a single-head linear-projection attention block.

Reference computation (B=4, CH=256, N=4096):
    theta = Wt @ x        [B, 32, N]
    phi   = Wp @ x        [B, 32, N]
    g     = Wg @ x        [B, 128, N]
    scores = theta^T phi  [B, N, N]
    beta = softmax(scores, axis=-1)
    attn = g @ beta^T     [B, 128, N]
    out = gamma * (Wo @ attn) + x

Sharding: 8 cores = 4 batches x 2 query-halves. Each core owns one batch's
full sequence (for keys/values) and half the queries. The per-core x is
rotated so its query half is always columns 0:2048, keeping the SPMD program
identical across cores (softmax/attention are invariant to a consistent
permutation of the key axis). No collectives are needed.

V2 design (per core, all matmuls bf16 with fp32 PSUM accumulation). The
kernel is Scalar-engine (exp) bound: 8.4M exps/core at 1 elem/cycle/lane on
the only engine with transcendentals. Everything is organized to keep ACT
~100% busy on maximal-width EXP instructions:
  - PSUM: one [128, 3, 1024] score "ring" (6 banks; slot = global m-tile
    index % 3, one m-tile x 1024 queries per slot, written as two 512-col
    matmuls per the one-bank rule) + one rotating 2-bank psA slot
    (prologue projections/gT, attn accumulation, epilogue) = 8 banks.
  - 2 passes of 1024 queries; per pass 32 m-tiles processed as 16 pairs.
    A pair's K=32 score matmuls run concurrently in distinct PE row groups
    (phi/theta replicated to partitions 0:64) into two ring slots; ONE
    [128, 2x1024] strided-AP EXP consumes both -> ~(2048+400)/1.2GHz
    ~ 2.0us per pair vs 2 x 1.2us unpaired. Slot pairs rotate
    (0,1),(2,0),(1,2) as constant-stride 2-element APs (negative stride
    for the wrap pair). ACT reads may span banks; matmul writes may not.
  - attn accumulated over m into psA [128, 2, 512] (two query-half bank
    groups); softmax denominator via a bf16 tree: leaf folds on Vector,
    level-1 + fp32 chain on the otherwise-idle GpSimd (GpSimd has no PSUM
    port, so it only ever touches SBUF), partition-reduced with one
    ones-matmul.
  - psA is shared sequentially: pass-0 drips proj/gT blocks between its
    first iterations, pass-1 drips pass-0's epilogue; attn matmuls start
    deferred with an expt backlog (expt lives in SBUF, decoupling attn
    from the ring). The final epilogue is column-chunked to shorten the
    serial tail.
  - prologue: x split over 4 DMA queues (weights first on the gpsimd
    queue, replicas on sync), projections start as soon as column-block 0
    lands, and a dense warm-up burst trips the PE clock monitor (HAM) to
    full rate. First EXP issues ~12us in (vs ~26us in V1).
  - gamma folded into Wo on the host; fp32 residual add with x.
"""

import os
import sys

import numpy as np

B, CH, N = 4, 256, 4096
NCORES = 8
NH = N // 2   # queries per core
P = 128
MT = N // P   # 32 m-tiles
NQ = 1024     # queries per pass
NPASS = NH // NQ  # 2

_REPO_CANDIDATES = ["/opt/trn_rl_repo", "/root/.axon_site/_ro/trn_rl_repo"]


def _ensure_import_path():
    try:
        import concourse.bass  # noqa: F401
        return
    except ImportError:
        pass
    for cand in _REPO_CANDIDATES:
        if os.path.isdir(cand):
            sys.path.insert(0, cand)
            try:
                import concourse.bass  # noqa: F401
                return
            except ImportError:
                sys.path.pop(0)
    raise ImportError("could not locate concourse (bass) repo")


_CACHE = {}


def build_bass():
    """Build + compile the per-core Tile program (identical on all 8 cores)."""
    _ensure_import_path()
    import concourse.bacc as bacc
    import concourse.tile as tile
    from concourse import mybir

    dt = mybir.dt
    f32 = dt.float32
    bf16 = dt.bfloat16
    Exp = mybir.ActivationFunctionType.Exp

    nc = bacc.Bacc(
        "TRN2",
        target_bir_lowering=False,
        debug=False,
        num_devices=NCORES,
    )

    # Per-core DRAM I/O.
    x_d = nc.dram_tensor("x", [CH, N], bf16, kind="ExternalInput")
    xq_d = nc.dram_tensor("xq", [CH, NH], f32, kind="ExternalInput")
    wt_d = nc.dram_tensor("wt", [CH, 32], bf16, kind="ExternalInput")   # Wt^T
    wp_d = nc.dram_tensor("wp", [CH, 32], bf16, kind="ExternalInput")   # Wp^T
    wg_d = nc.dram_tensor("wg", [CH, 128], bf16, kind="ExternalInput")  # Wg^T
    wo_d = nc.dram_tensor("wo", [128, CH], bf16, kind="ExternalInput")  # (gamma*Wo)^T
    out_d = nc.dram_tensor("out", [CH, NH], f32, kind="ExternalOutput")

    with tile.TileContext(nc) as tc:
        with (
            tc.tile_pool(name="const", bufs=1) as const,
            tc.tile_pool(name="xp", bufs=1) as xp,
            tc.tile_pool(name="proj", bufs=1) as proj,
            tc.tile_pool(name="expp", bufs=14) as expp,
            tc.tile_pool(name="tree", bufs=6) as tree,
            tc.tile_pool(name="acc", bufs=2) as acc,
            tc.tile_pool(name="outp", bufs=1) as outp,
            tc.tile_pool(name="ringp", bufs=1, space="PSUM") as ringp,
            tc.tile_pool(name="psA", bufs=1, space="PSUM") as psA,
        ):
            # ---- tiny weights first, on the gpsimd HWDGE queue ----
            wt_sb = const.tile([P, 2, 32], bf16)
            wp_sb = const.tile([P, 2, 32], bf16)
            wg_sb = const.tile([P, 2, 128], bf16)
            wo_sb = const.tile([P, CH], bf16)
            ones_sb = const.tile([P, P], bf16)
            warm_sb = const.tile([P, 512], bf16)
            nc.gpsimd.dma_start(
                out=wt_sb, in_=wt_d.ap().rearrange("(kb p) m -> p kb m", p=P)
            )
            nc.gpsimd.dma_start(
                out=wp_sb, in_=wp_d.ap().rearrange("(kb p) m -> p kb m", p=P)
            )
            nc.gpsimd.dma_start(
                out=wg_sb, in_=wg_d.ap().rearrange("(kb p) m -> p kb m", p=P)
            )
            nc.vector.memset(ones_sb, 1.0)
            nc.vector.memset(warm_sb, 0.0)

            # ---- x column blocks across 4 queues; cb0 (both kb) first ----
            x_sb = xp.tile([P, 2, N], bf16)
            xq_sb = xp.tile([P, 2, NH], f32)

            def dma_x(eng, kb, cb):
                eng.dma_start(
                    out=x_sb[:, kb, cb * 1024:(cb + 1) * 1024],
                    in_=x_d[kb * P:(kb + 1) * P, cb * 1024:(cb + 1) * 1024],
                )

            for cb in range(4):
                dma_x(nc.sync, 0, cb)
                dma_x(nc.scalar, 1, cb)
            nc.gpsimd.dma_start(out=wo_sb, in_=wo_d.ap())
            # residual slice: only needed by the epilogues (first ~45us in);
            # gpsimd queue so its long transfer never delays x or replicas
            for kb in range(2):
                nc.gpsimd.dma_start(
                    out=xq_sb[:, kb, :], in_=xq_d[kb * P:(kb + 1) * P, :]
                )

            # ---- PSUM: 3-slot fp32 score ring (6 banks; slot = one m-tile
            # x 1024 queries, written as two 512-col matmuls per the
            # one-bank rule, consumed by one single-slot EXP) + psA ----
            ring = ringp.tile([P, 3, NQ], f32)

            # dense dummy matmul burst during the initial x-DMA wait: trips
            # the PE clock monitor (HAM) to full rate before the projections
            for _ in range(12):
                nc.tensor.matmul(
                    ring[:, 2, 0:512], lhsT=ones_sb, rhs=warm_sb,
                    start=True, stop=True,
                )

            # ---- projections: theta/phi column-packed per 1024-col block
            # (theta at psum partitions 0:32, phi at 32:64), one cast each,
            # then replica DMAs (sync queue) fill the opposite 32-partition
            # group so both live at 0:64 for 2x row-group score packing. ----
            theta_sb = proj.tile([64, NH], bf16)
            phi_sb = proj.tile([64, N], bf16)
            gT_sb = proj.tile([P, MT, P], bf16)

            def emit_proj(cb):
                cbs = slice(cb * 1024, (cb + 1) * 1024)
                ps_p = psA.tile([64, 2, 512], f32, tag="ps")
                for h in range(2):
                    chs = slice(cb * 1024 + h * 512, cb * 1024 + (h + 1) * 512)
                    for kb in range(2):
                        if cb < 2:
                            nc.tensor.matmul(
                                ps_p[0:32, h, :],
                                lhsT=wt_sb[:, kb, :],
                                rhs=x_sb[:, kb, chs],
                                start=(kb == 0),
                                stop=(kb == 1),
                                skip_group_check=True,
                            )
                        nc.tensor.matmul(
                            ps_p[32:64, h, :],
                            lhsT=wp_sb[:, kb, :],
                            rhs=x_sb[:, kb, chs],
                            start=(kb == 0),
                            stop=(kb == 1),
                            skip_group_check=True,
                        )
                if cb < 2:
                    nc.vector.tensor_copy(
                        out=theta_sb[0:32, cbs].rearrange("p (h c) -> p h c", h=2),
                        in_=ps_p[0:32, :, :],
                    )
                    nc.sync.dma_start(
                        out=theta_sb[32:64, cbs], in_=theta_sb[0:32, cbs]
                    )
                nc.vector.tensor_copy(
                    out=phi_sb[32:64, cbs].rearrange("p (h c) -> p h c", h=2),
                    in_=ps_p[32:64, :, :],
                )
                nc.sync.dma_start(out=phi_sb[0:32, cbs], in_=phi_sb[32:64, cbs])

            def emit_gt_group(grp):
                # gT[m, c] for 8 m-tiles; cast back to SBUF on Vector
                # (GpSimd cannot read PSUM)
                ps_g = psA.tile([P, 8, P], f32, tag="ps")
                for j in range(8):
                    mt = grp * 8 + j
                    for kb in range(2):
                        nc.tensor.matmul(
                            ps_g[:, j, :],
                            lhsT=x_sb[:, kb, mt * P:(mt + 1) * P],
                            rhs=wg_sb[:, kb, :],
                            start=(kb == 0),
                            stop=(kb == 1),
                        )
                nc.vector.tensor_copy(
                    out=gT_sb[:, grp * 8:(grp + 1) * 8, :], in_=ps_g
                )

            # block 0 before the loop: everything iteration 0 needs
            emit_proj(0)
            emit_gt_group(0)

            out_sb = outp.tile([P, 2, NH], f32)

            # ---------------- epilogue pieces (generator) ----------------
            def epilogue_pieces(nh, attn_ps, root_bf, nchunk=1):
                """Normalization + output projection for pass nh. Piece 0
                (the cast) frees the attn psA slot; later pieces reuse the
                psA banks sequentially. root_bf: [P, NQ] bf16 per-partition
                partial denominators (pre partition-reduction)."""
                A_bf = acc.tile([P, NQ], bf16, tag="abf")
                nc.vector.tensor_copy(
                    out=A_bf.rearrange("p (h c) -> p h c", h=2), in_=attn_ps
                )
                yield
                ps_S = psA.tile([P, 2, 512], f32, tag="ps")
                for h in range(2):
                    nc.tensor.matmul(
                        ps_S[:, h, :],
                        lhsT=ones_sb,
                        rhs=root_bf[:, h * 512:(h + 1) * 512],
                        start=True,
                        stop=True,
                        skip_group_check=True,
                    )
                yield
                recip = acc.tile([P, NQ], f32, tag="recip")
                nc.vector.reciprocal_approx_fast(
                    out=recip.rearrange("p (h c) -> p h c", h=2), in_=ps_S
                )
                yield
                CW = 512
                for ck in range(NQ // CW):
                    cks = slice(ck * CW, (ck + 1) * CW)
                    gks = slice(nh * NQ + ck * CW, nh * NQ + (ck + 1) * CW)
                    ps_o = psA.tile([P, 2, 512], f32, tag="ps")
                    for oc in range(2):
                        nc.tensor.matmul(
                            ps_o[:, oc, :],
                            lhsT=wo_sb[:, oc * P:(oc + 1) * P],
                            rhs=A_bf[:, cks],
                            start=True,
                            stop=True,
                            skip_group_check=True,
                        )
                    for oc in range(2):
                        tmp = acc.tile([P, CW], f32, tag=f"tmp{oc}")
                        nc.vector.tensor_mul(tmp, ps_o[:, oc, :], recip[:, cks])
                        nc.gpsimd.tensor_add(
                            out_sb[:, oc, gks], tmp, xq_sb[:, oc, gks]
                        )
                        nc.sync.dma_start(
                            out=out_d[oc * P:(oc + 1) * P, gks],
                            in_=out_sb[:, oc, gks],
                        )
                        if nchunk > 1:
                            yield
                    if nchunk == 1 and ck == NQ // CW - 1:
                        yield

            # -------------------- main loop: 2 passes --------------------
            # psA slot owners are strictly sequential: proj0,gT0 (above),
            # then pass-0 drips proj1,gT1,...,proj3,gT3, then attn(p0),
            # then pass-0's epilogue dripped into pass 1, then attn(p1), ...
            pending = None
            leftover = []  # previous pass's lagged attn matmuls
            for nh in range(NPASS):
                qs = nh * NQ
                if nh == 0:
                    drip = []
                    for cb in range(1, 4):
                        drip.append(lambda c=cb: emit_proj(c))
                        drip.append(lambda c=cb: emit_gt_group(c))
                else:
                    drip = None  # pass>0: drip = previous epilogue pieces

                def flush_prev_attn():
                    # lagged attn matmuls of the PREVIOUS pass: their exps
                    # are long done, so these never stall the PE queue
                    nonlocal leftover
                    for mt, et, aps, st, sp in leftover:
                        for h in range(2):
                            nc.tensor.matmul(
                                aps[:, h, :],
                                lhsT=gT_sb[:, mt, :],
                                rhs=et[:, h * 512:(h + 1) * 512],
                                start=st,
                                stop=sp,
                                skip_group_check=True,
                            )
                    leftover = []

                attn_ps = None
                attn_cnt = 0
                attn_backlog = []  # (mt, expt_tile, half) awaiting emission
                quad = None
                chain = None
                for k in range(MT // 2):
                    mtE, mtO = 2 * k, 2 * k + 1
                    tg = nh * MT + 2 * k  # global tile counter
                    sE, sO = tg % 3, (tg + 1) % 3
                    # --- lagged attn matmuls first: they are >=3 pairs old,
                    #     so they never block the PE queue on a recent exp ---
                    keep = 4 if not (nh == NPASS - 1 and k >= MT // 2 - 2) else 2
                    if attn_ps is not None:
                        while len(attn_backlog) > keep:
                            mt, et = attn_backlog.pop(0)
                            for h in range(2):
                                nc.tensor.matmul(
                                    attn_ps[:, h, :],
                                    lhsT=gT_sb[:, mt, :],
                                    rhs=et[:, h * 512:(h + 1) * 512],
                                    start=(attn_cnt == 0),
                                    stop=False,
                                    skip_group_check=True,
                                )
                            attn_cnt += 1
                    # --- score matmuls: two 512-col halves per m-tile (the
                    #     one-bank rule); the two m-tiles of the pair run in
                    #     concurrent PE row groups ---
                    for h in range(2):
                        for j, (mt, s) in enumerate(((mtE, sE), (mtO, sO))):
                            nc.tensor.matmul(
                                ring[:, s, h * 512:(h + 1) * 512],
                                lhsT=phi_sb[
                                    32 * j:32 * (j + 1), mt * P:(mt + 1) * P
                                ],
                                rhs=theta_sb[
                                    32 * j:32 * (j + 1),
                                    qs + h * 512:qs + (h + 1) * 512,
                                ],
                                start=True,
                                stop=True,
                                skip_group_check=True,
                            )
                    # --- one single-slot exp per m-tile (the 3-slot ring
                    #     keeps the producer ~2 tiles ahead) ---
                    etE = expp.tile([P, NQ], bf16, tag="expt")
                    nc.scalar.activation(out=etE, in_=ring[:, sE, :], func=Exp)
                    etO = expp.tile([P, NQ], bf16, tag="expt")
                    nc.scalar.activation(out=etO, in_=ring[:, sO, :], func=Exp)
                    if k == 0:
                        flush_prev_attn()
                    attn_backlog.append((mtE, etE))
                    attn_backlog.append((mtO, etO))
                    # --- drip prologue blocks / previous pass's epilogue;
                    #     attn matmuls open once psA's queue frees up, and
                    #     keep two tiles of lag so they never wait on the
                    #     exp that was just issued ---
                    if drip:
                        drip.pop(0)()
                    elif pending is not None:
                        if next(pending, "done") == "done":
                            pending = None
                    if not drip and pending is None and attn_ps is None:
                        attn_ps = psA.tile([P, 2, 512], f32, tag="ps")
                    # --- denominator tree: leaf folds on Vector, level-1
                    #     and fp32 chain on GpSimd (SBUF-only), bf16 root ---
                    leaf = tree.tile([P, NQ], bf16, tag="leaf")
                    nc.vector.tensor_add(leaf, etE, etO)
                    if quad is None:
                        quad = leaf
                    else:
                        node = tree.tile([P, NQ], bf16, tag="lvl1")
                        nc.vector.tensor_add(node, quad, leaf)
                        quad = None
                        last = k == MT // 2 - 1
                        if chain is None:
                            chain = node
                        else:
                            nt = tree.tile(
                                [P, NQ], bf16 if last else f32,
                                tag="sroot" if last else "chain",
                            )
                            nc.gpsimd.tensor_add(nt, chain, node)
                            chain = nt
                # hand this pass's lagged attn tail to the next pass (or to
                # the post-loop flush for the final pass)
                assert quad is None and pending is None and not drip
                assert attn_cnt + len(attn_backlog) == MT
                nb = len(attn_backlog)
                for i, (mt, et) in enumerate(attn_backlog):
                    leftover.append(
                        (mt, et, attn_ps, attn_cnt == 0 and i == 0, i == nb - 1)
                    )
                attn_backlog = []
                pending = epilogue_pieces(
                    nh, attn_ps, chain, nchunk=1 if nh < NPASS - 1 else 2
                )
            flush_prev_attn()
            for _ in pending:
                pass

    nc.compile()
    return nc


def get_nc():
    if "nc" not in _CACHE:
        _CACHE["nc"] = build_bass()
    return _CACHE["nc"]


def make_in_maps(x, Wt, Wp, Wg, Wo, gamma):
    import ml_dtypes

    bf16 = ml_dtypes.bfloat16
    x = np.asarray(x, dtype=np.float32)
    wt = np.ascontiguousarray(np.asarray(Wt, np.float32).T).astype(bf16)
    wp = np.ascontiguousarray(np.asarray(Wp, np.float32).T).astype(bf16)
    wg = np.ascontiguousarray(np.asarray(Wg, np.float32).T).astype(bf16)
    wo = np.ascontiguousarray(
        (float(np.asarray(gamma)) * np.asarray(Wo, np.float32)).T
    ).astype(bf16)
    in_maps = []
    for i in range(NCORES):
        b, h = divmod(i, 2)
        xb = x[b]
        if h:
            xb = np.concatenate([xb[:, NH:], xb[:, :NH]], axis=1)
        in_maps.append(
            {
                "x": np.ascontiguousarray(xb).astype(bf16),
                "xq": np.ascontiguousarray(x[b][:, h * NH:(h + 1) * NH]),
                "wt": wt,
                "wp": wp,
                "wg": wg,
                "wo": wo,
            }
        )
    return in_maps


def gather_out(results):
    out = np.empty((B, CH, N), np.float32)
    for i in range(NCORES):
        b, h = divmod(i, 2)
        out[b][:, h * NH:(h + 1) * NH] = results[i]["out"]
    return out


def kernel(x, Wt, Wp, Wg, Wo, gamma):
    _ensure_import_path()
    from concourse.bass_utils import run_bass_kernel_spmd

    nc = get_nc()
    in_maps = make_in_maps(x, Wt, Wp, Wg, Wo, gamma)
    res = run_bass_kernel_spmd(nc, in_maps, core_ids=list(range(NCORES)))
    return gather_out(res.results)
